# revision 17
# baseline (speedup 1.0000x reference)
"""Trainium2 Bass kernel for MessagePassingLayerV1 (bf16 dual-batch design).

Reference computation (per batch b):
    h_self = h @ W_self.T
    msg    = h[:, src, :] @ W_msg.T               (per edge)
    h_agg[n] = mean over {e: dst[e]==n} of msg[e]  (count clamped >= 1)
    x = h + relu(h_self + h_agg + bias)
    out = LayerNorm(x) * gamma + beta

Key restructures vs the fp32 baseline:
  * W_msg applied AFTER the mean (linearity), so only raw h[src] is gathered.
  * Both batches share edge_index, so each node's features for BOTH batches
    are interleaved into one bf16 row of 512B: ONE dma_gather descriptor per
    edge covers both batches (half the descriptors, half the bytes of the
    fp32 single-batch scheme; 512B is the DMA full-rate boundary).
  * All matmuls bf16: 1 cycle/row vs fp32's 4 (tolerance is 2e-2).
  * Scatter-add via matmul: per 128-edge tile, aggT[f,dst] += X.T @ S with
    S[e,j] = (iota[j] == slot[e]) * (1/count[dst[e]]) built per tile by one
    DVE tensor_scalar (is_equal, mult) — bf16 in/out with f32 per-partition
    scalars keeps the 4x_2p DVE fast path. aggT copies PSUM->SBUF on the ACT
    engine (Copy, bf16 out).
  * h rows for the residual are NOT loaded: they are recovered on-chip by
    PE-transposing the (needed anyway) hT tiles into PSUM; the relu+residual
    DVE op reads them straight from PSUM.
  * LayerNorm stats for 4 windows x 2 batches are packed into [128,8] tiles
    so the small-op chain runs once per 4 windows; eps dropped (pad rows get
    a +-1e-3 pattern so var >= ~1e-6; relative effect < 1e-5 on real rows);
    y emitted bf16 on ACT via Identity(x*rstd - mu*rstd).

Sharding: 8 cores x (1/8 of dst nodes, BOTH batches). Single SPMD program:
per-(window, half) tile counts are padded to the max across the 8 groups.
No collectives; host assembles the 8 disjoint output shards.
"""

import sys
from contextlib import ExitStack

import numpy as np

sys.path.insert(0, "/opt/trn_rl_repo")

import ml_dtypes  # noqa: E402

import concourse.bacc as bacc  # noqa: E402
import concourse.bass as bass  # noqa: E402
import concourse.mybir as mybir  # noqa: E402
import concourse.tile as tile  # noqa: E402
from concourse._compat import get_trn_type as _get_trn_type  # noqa: E402
from concourse.bass_utils import run_bass_kernel_spmd  # noqa: E402
from concourse.library_config import mlp as _mlp_library  # noqa: E402

F32 = mybir.dt.float32
BF16 = mybir.dt.bfloat16
I16 = mybir.dt.int16
ALU = mybir.AluOpType
ACTF = mybir.ActivationFunctionType
NPBF = ml_dtypes.bfloat16

PAD_DLOC = 200.0  # dst_local sentinel: never equals iota 0..127 -> S row = 0

# Full-problem geometry (hardcoded per harness contract).
FULL_GEO = dict(B=2, N=50000, D=128, NG=8, NW_EXTRA=0)

GMAX = 8  # gather tiles (128 idx each) per dma_gather call (1024-idx ucode cap)
SCRATCH = 65536  # SWDGE descriptor ring: 4096 descs = 4 calls in flight
CHW = 10  # hT chunk size in windows
GLN = 8  # windows per LayerNorm stats batch

# Holder for the last run's BassKernelResults (test.py reads exec_time_ns).
LAST_RESULTS = None


def _geometry(g):
    B, N, NG = g["B"], g["N"], g["NG"]
    n_core = N // NG
    assert n_core * NG == N
    nw = -(-n_core // 128) + g.get("NW_EXTRA", 0)
    half = N // 2
    return B, N, g["D"], NG, n_core, nw, half


def _preprocess(edge_index, g):
    """Per-group edge metadata, padded to uniform tile counts across groups.

    Nodes are assigned to 128-slot windows with a degree-balanced greedy so
    per-window-half edge counts are nearly equal across windows AND groups.
    Returns (T_low, T_high, cl, ch, per_group); per_group[q] has ixlo/ixhi
    (wrapped int16), dllo/dlhi + crlo/crhi (f32 [128,T]), perm ([nw*128]
    global node id per slot, -1 = pad).
    """
    B, N, D, NG, n_core, nw, half = _geometry(g)
    n_pad = nw * 128
    src = np.asarray(edge_index[0]).astype(np.int64)
    dst = np.asarray(edge_index[1]).astype(np.int64)
    counts = np.bincount(dst, minlength=N).astype(np.float32)
    crec_node = (1.0 / np.maximum(counts, 1.0)).astype(np.float32)

    groups = {}
    nlow = np.zeros((NG, nw), np.int64)
    nhigh = np.zeros((NG, nw), np.int64)
    perms = []
    lo_edge = src < half
    degs = []
    for q in range(NG):
        base = q * n_core
        qsel = (dst >= base) & (dst < base + n_core)
        dloc_all = dst[qsel] - base
        deg_lo = np.bincount(dloc_all[lo_edge[qsel]], minlength=n_core)
        deg_hi = np.bincount(dloc_all[~lo_edge[qsel]], minlength=n_core)
        degs.append((deg_lo, deg_hi))
    base_tiles = max(
        1,
        int(np.ceil(max(max(dl.sum(), dh.sum()) for dl, dh in degs) / nw / 128)),
    )
    cap0 = 128 * base_tiles
    nspill = [
        int(np.ceil(max(0.0, max(dl.sum(), dh.sum()) - cap0 * nw) / 128))
        for dl, dh in degs
    ]
    nspill_max = max(nspill)
    caps = np.full(nw, cap0)
    caps[:nspill_max] = cap0 + 128
    for q in range(NG):
        base = q * n_core
        deg_lo, deg_hi = degs[q]
        order = np.argsort(-(deg_lo + deg_hi), kind="stable")
        n_lo = np.zeros(nw)
        n_hi = np.zeros(nw)
        fill = np.zeros(nw, np.int64)
        wof = np.empty(n_core, np.int64)
        slot = np.empty(n_core, np.int64)
        perm = np.full(n_pad, -1, np.int64)
        tcap = caps / 128.0
        for nl in order:
            a = n_lo + deg_lo[nl]
            b = n_hi + deg_hi[nl]
            pen = (
                np.maximum(np.ceil(a / 128.0) - tcap, 0)
                - np.maximum(np.ceil(n_lo / 128.0) - tcap, 0)
                + np.maximum(np.ceil(b / 128.0) - tcap, 0)
                - np.maximum(np.ceil(n_hi / 128.0) - tcap, 0)
            )
            score = np.maximum(a, b) + 1e6 * pen
            score[fill >= 128] = np.inf
            w = int(np.argmin(score))
            wof[nl] = w
            slot[nl] = fill[w]
            perm[w * 128 + fill[w]] = base + nl
            fill[w] += 1
            n_lo[w] += deg_lo[nl]
            n_hi[w] += deg_hi[nl]
        assert fill.max() <= 128
        perms.append(perm)

        sel = (dst >= base) & (dst < base + n_core)
        s_q = src[sel]
        d_loc = dst[sel] - base
        w_e = wof[d_loc]
        o1 = np.lexsort((s_q, w_e))
        s_q, d_loc, w_e = s_q[o1], d_loc[o1], w_e[o1]
        bounds = np.searchsorted(w_e, np.arange(nw + 1))
        for w in range(nw):
            sw = s_q[bounds[w] : bounds[w + 1]]
            dw = d_loc[bounds[w] : bounds[w + 1]]
            lo = sw < half
            for tag, mask, sbase in (("lo", lo, 0), ("hi", ~lo, half)):
                s_g = sw[mask] - sbase
                d_g = dw[mask]
                o2 = np.argsort(s_g, kind="stable")
                groups[(q, w, tag)] = (
                    s_g[o2],
                    slot[d_g[o2]].astype(np.float32),
                    crec_node[d_g[o2] + base],
                )
                if tag == "lo":
                    nlow[q, w] = s_g.size
                else:
                    nhigh[q, w] = s_g.size

    T_low = -(-nlow.max(axis=0) // 128)
    T_high = -(-nhigh.max(axis=0) // 128)
    empty = (T_low + T_high) == 0
    T_low[empty] = 1
    cl = np.concatenate([[0], np.cumsum(T_low)]).astype(np.int64)
    ch = np.concatenate([[0], np.cumsum(T_high)]).astype(np.int64)

    def wrap_idx(arr):
        # dma_gather layout: idx j -> partition j%16, col j//16; replicated x8.
        a = arr.reshape(-1, 16).T.astype(np.int16)
        return np.ascontiguousarray(np.tile(a, (8, 1)))

    per_group = []
    for q in range(NG):
        out = {}
        for tag, T, cum in (("lo", T_low, cl), ("hi", T_high, ch)):
            tot = int(cum[-1])
            idx = np.zeros(tot * 128, np.int64)
            dl = np.full(tot * 128, PAD_DLOC, np.float32)
            cr = np.zeros(tot * 128, np.float32)
            for w in range(nw):
                s_g, d_g, c_g = groups[(q, w, tag)]
                off = int(cum[w]) * 128
                idx[off : off + s_g.size] = s_g
                dl[off : off + s_g.size] = d_g
                cr[off : off + s_g.size] = c_g
            out["ix" + tag] = wrap_idx(idx)
            out["dl" + tag] = np.ascontiguousarray(dl.reshape(tot, 128).T)
            out["cr" + tag] = np.ascontiguousarray(cr.reshape(tot, 128).T)
        out["perm"] = perms[q]
        per_group.append(out)
    return T_low, T_high, cl, ch, per_group


def _build_program(g, T_low, T_high, cl, ch, trivial=(True, True, True)):
    B, N, D, NG, n_core, nw, half = _geometry(g)
    TL, TH = int(cl[-1]), int(ch[-1])
    triv_bias, triv_gamma, triv_beta = trivial
    D2 = 2 * D

    nc = bacc.Bacc(
        _get_trn_type() or "TRN2",
        target_bir_lowering=False,
        debug=False,
        num_devices=NG,
        dynamic_dma_scratch_size=SCRATCH,
    )
    d_hlo = nc.dram_tensor("h2lo", [half, D2], BF16, kind="ExternalInput")
    d_hhi = nc.dram_tensor("h2hi", [N - half, D2], BF16, kind="ExternalInput")
    d_ixlo = nc.dram_tensor("ixlo", [128, TL * 8], I16, kind="ExternalInput")
    d_ixhi = nc.dram_tensor("ixhi", [128, TH * 8], I16, kind="ExternalInput")
    d_dllo = nc.dram_tensor("dllo", [128, TL], F32, kind="ExternalInput")
    d_dlhi = nc.dram_tensor("dlhi", [128, TH], F32, kind="ExternalInput")
    d_crlo = nc.dram_tensor("crlo", [128, TL], F32, kind="ExternalInput")
    d_crhi = nc.dram_tensor("crhi", [128, TH], F32, kind="ExternalInput")
    d_hT = nc.dram_tensor("hT", [128, nw, 2, D], BF16, kind="ExternalInput")
    d_wsT = nc.dram_tensor("wsT", [D, D], BF16, kind="ExternalInput")
    d_wmT = nc.dram_tensor("wmT", [D, D], BF16, kind="ExternalInput")
    d_iota = nc.dram_tensor("iota", [128, 128], BF16, kind="ExternalInput")
    d_ident = nc.dram_tensor("ident", [128, 128], BF16, kind="ExternalInput")
    d_bias = d_gam = d_bet = None
    if not triv_bias:
        d_bias = nc.dram_tensor("bias_b", [128, D], F32, kind="ExternalInput")
    if not triv_gamma:
        d_gam = nc.dram_tensor("gamma_b", [128, D], F32, kind="ExternalInput")
    if not triv_beta:
        d_bet = nc.dram_tensor("beta_b", [128, D], F32, kind="ExternalInput")
    d_out = nc.dram_tensor("out", [128, nw, 2, D], BF16, kind="ExternalOutput")

    with tile.TileContext(nc) as tc, ExitStack() as ctx:
        cpool = ctx.enter_context(tc.tile_pool(name="const", bufs=1))
        gplo = ctx.enter_context(tc.tile_pool(name="glo", bufs=4))
        gphi = ctx.enter_context(tc.tile_pool(name="ghi", bufs=4))
        htp = ctx.enter_context(tc.tile_pool(name="htp", bufs=2))
        spool = ctx.enter_context(tc.tile_pool(name="sel", bufs=8))
        apool = ctx.enter_context(tc.tile_pool(name="aggts", bufs=4))
        xpool = ctx.enter_context(tc.tile_pool(name="xp", bufs=GLN + 2))
        sqpool = ctx.enter_context(tc.tile_pool(name="sqp", bufs=2))
        ypool = ctx.enter_context(tc.tile_pool(name="yp", bufs=6))
        stpool = ctx.enter_context(tc.tile_pool(name="stats", bufs=2))
        ppA = ctx.enter_context(
            tc.tile_pool(name="psA", bufs=3, space=bass.MemorySpace.PSUM)
        )
        ppB = ctx.enter_context(
            tc.tile_pool(name="psB", bufs=3, space=bass.MemorySpace.PSUM)
        )
        ppT = ctx.enter_context(
            tc.tile_pool(name="psT", bufs=2, space=bass.MemorySpace.PSUM)
        )

        nc.gpsimd.load_library(_mlp_library)

        def cload(dram, shape, dtype=BF16):
            t = cpool.tile(shape, dtype, tag=dram.name, name=dram.name + "_t")
            nc.sync.dma_start(t[:], dram[:])
            return t

        t_ixlo = cload(d_ixlo, [128, TL * 8], I16)
        t_dllo = cload(d_dllo, [128, TL], F32)
        t_crlo = cload(d_crlo, [128, TL], F32)
        t_ixhi = cload(d_ixhi, [128, TH * 8], I16)
        t_dlhi = cload(d_dlhi, [128, TH], F32)
        t_crhi = cload(d_crhi, [128, TH], F32)
        t_iota = cload(d_iota, [128, 128])
        t_ident = cload(d_ident, [128, 128])
        t_wsT = cload(d_wsT, [D, D])
        t_wmT = cload(d_wmT, [D, D])
        t_bias = None if triv_bias else cload(d_bias, [128, D], F32)
        t_gam = None if triv_gamma else cload(d_gam, [128, D], F32)
        t_bet = None if triv_beta else cload(d_bet, [128, D], F32)

        blocks = {"lo": [], "hi": []}
        issued = {"lo": 0, "hi": 0}
        totals = {"lo": TL, "hi": TH}
        gsrc = {"lo": d_hlo, "hi": d_hhi}
        gix = {"lo": t_ixlo, "hi": t_ixhi}
        gpool = {"lo": gplo, "hi": gphi}

        def ensure_gathered(kind, upto):
            while issued[kind] < min(upto, totals[kind]):
                t0 = issued[kind]
                t1 = min(t0 + GMAX, totals[kind])
                blk = gpool[kind].tile(
                    [128, GMAX, D2], BF16, tag="g" + kind, name="g" + kind
                )
                nc.gpsimd.dma_gather(
                    blk[:, 0 : t1 - t0, :],
                    gsrc[kind][:],
                    gix[kind][:, t0 * 8 : t1 * 8],
                    num_idxs=(t1 - t0) * 128,
                    num_idxs_reg=(t1 - t0) * 128,
                    elem_size=D2,
                )
                blocks[kind].append(blk)
                issued[kind] = t1

        ht_tiles = {}
        pending = []  # (w, x_tile) awaiting group LayerNorm stats
        xs_g = ssq_g = None

        def flush_group(wlist):
            nonlocal xs_g, ssq_g
            gw = len(wlist)
            mu = stpool.tile([128, 2 * GLN], F32, tag="mu", name="mu")
            nc.vector.tensor_scalar_mul(mu[:, 0 : 2 * gw], xs_g[:, 0 : 2 * gw], 1.0 / D)
            nmusq = stpool.tile([128, 2 * GLN], F32, tag="nmusq", name="nmusq")
            nc.vector.scalar_tensor_tensor(
                nmusq[:, 0 : 2 * gw],
                mu[:, 0 : 2 * gw],
                -1.0,
                mu[:, 0 : 2 * gw],
                op0=ALU.mult,
                op1=ALU.mult,
            )
            varv = stpool.tile([128, 2 * GLN], F32, tag="varv", name="varv")
            nc.vector.scalar_tensor_tensor(
                varv[:, 0 : 2 * gw],
                ssq_g[:, 0 : 2 * gw],
                1.0 / D,
                nmusq[:, 0 : 2 * gw],
                op0=ALU.mult,
                op1=ALU.add,
            )
            vrec = stpool.tile([128, 2 * GLN], F32, tag="vrec", name="vrec")
            nc.vector.reciprocal(vrec[:, 0 : 2 * gw], varv[:, 0 : 2 * gw])
            rstd = stpool.tile([128, 2 * GLN], F32, tag="rstd", name="rstd")
            nc.scalar.activation(rstd[:, 0 : 2 * gw], vrec[:, 0 : 2 * gw], ACTF.Sqrt)
            nmr = stpool.tile([128, 2 * GLN], F32, tag="nmr", name="nmr")
            nc.vector.scalar_tensor_tensor(
                nmr[:, 0 : 2 * gw],
                mu[:, 0 : 2 * gw],
                -1.0,
                rstd[:, 0 : 2 * gw],
                op0=ALU.mult,
                op1=ALU.mult,
            )
            for wi, (w2, x2) in enumerate(wlist):
                y = ypool.tile([128, 2, D], BF16, tag="y", name="y")
                for b in (0, 1):
                    c = wi * 2 + b
                    yt = y[:, b, :]
                    if not (triv_gamma and triv_beta):
                        yf = ypool.tile([128, D], F32, tag="yf", name="yf")
                        yt = yf[:]
                    nc.scalar.activation(
                        yt,
                        x2[:, b, :],
                        ACTF.Identity,
                        bias=nmr[:, c : c + 1],
                        scale=rstd[:, c : c + 1],
                    )
                    if not triv_gamma:
                        yg = ypool.tile([128, D], F32, tag="yg", name="yg")
                        nc.vector.tensor_mul(yg[:], yt, t_gam[:])
                        yt = yg[:]
                    if not triv_beta:
                        nc.vector.tensor_add(y[:, b, :], yt, t_bet[:])
                    elif not triv_gamma:
                        nc.vector.tensor_copy(y[:, b, :], yt)
                nc.sync.dma_start(d_out[:, w2, :, :], y[:])

        for w in range(nw):
            tlc = int(T_low[w])
            thc = int(T_high[w])
            cl0, ch0 = int(cl[w]), int(ch[w])
            ensure_gathered("lo", cl0 + tlc)
            ensure_gathered("hi", ch0 + thc)

            ci = w // CHW
            want = [ci, ci + 1] if w % CHW >= CHW - 3 else [ci]
            for cli in want:
                if cli not in ht_tiles and cli * CHW < nw:
                    c0 = cli * CHW
                    c1 = min(c0 + CHW, nw)
                    tht2 = htp.tile([128, CHW, 2, D], BF16, tag="ht", name="ht")
                    nc.sync.dma_start(
                        tht2[:, 0 : c1 - c0, :, :], d_hT[:, c0:c1, :, :]
                    )
                    ht_tiles[cli] = tht2
            tht = ht_tiles[ci]
            k = w % CHW

            if w % GLN == 0:
                xs_g = stpool.tile([128, 2 * GLN], F32, tag="xs", name="xs")
                ssq_g = stpool.tile([128, 2 * GLN], F32, tag="ssq", name="ssq")
            gcol0 = 2 * (w % GLN)

            # S tiles: one-hot * 1/count, one fused DVE tensor_scalar per tile
            # (bf16 in/out + per-partition f32 scalars -> 4x DVE mode).
            ntile = tlc + thc
            S = spool.tile([128, ntile, 128], BF16, tag="S", name="S")
            tiles = [("lo", cl0 + kk, kk, t_dllo, t_crlo) for kk in range(tlc)] + [
                ("hi", ch0 + kk, tlc + kk, t_dlhi, t_crhi) for kk in range(thc)
            ]
            for kind, t, scol, t_dl, t_cr in tiles:
                nc.vector.tensor_scalar(
                    S[:, scol, :],
                    t_iota[:],
                    t_dl[:, t : t + 1],
                    t_cr[:, t : t + 1],
                    op0=ALU.is_equal,
                    op1=ALU.mult,
                )

            # aggT[f, dst] per batch, accumulated over edge tiles in PSUM.
            # Batch groups run sequentially so both fit one PSUM bank.
            psA = ppA.tile([128, 2, 128], F32, tag="psA", name="psA")
            for b in (0, 1):
                for j, (kind, t, scol, _dl, _cr) in enumerate(tiles):
                    blk = blocks[kind][t // GMAX]
                    slot = t % GMAX
                    nc.tensor.matmul(
                        psA[:, b, :],
                        blk[:, slot, b * D : (b + 1) * D],
                        S[:, scol, :],
                        start=j == 0,
                        stop=j == len(tiles) - 1,
                    )

            # PSUM -> SBUF bf16 copy of aggT on the (otherwise idle) ACT engine.
            aggTs = apool.tile([128, 2, 128], BF16, tag="aggTs", name="aggTs")
            nc.vector.tensor_copy(aggTs[:], psA[:])

            # h rows (residual) recovered on-chip: hrowT = transpose(hT).
            # Vector ops may read only ONE PSUM input (psB takes that slot),
            # so stage the transposed rows to SBUF, one copy per engine.
            hrT = ppT.tile([128, 2, D], BF16, tag="hrT", name="hrT")
            for b in (0, 1):
                nc.tensor.transpose(hrT[:, b, :], tht[:, k, b, :], t_ident[:])
            hrs = apool.tile([128, 2, D], BF16, tag="hrs", name="hrs")
            nc.scalar.activation(hrs[:], hrT[:], ACTF.Copy)

            # psB[node, fo] = h @ W_self.T + h_agg @ W_msg.T  (per batch)
            psB = ppB.tile([128, 2, 128], F32, tag="psB", name="psB")
            for b in (0, 1):
                nc.tensor.matmul(
                    psB[:, b, :], tht[:, k, b, :], t_wsT[:], start=True, stop=False
                )
                nc.tensor.matmul(
                    psB[:, b, :], aggTs[:, b, :], t_wmT[:], start=False, stop=True
                )

            # x = h + relu(psB [+ bias]); row-sum accum into the group tile.
            x = xpool.tile([128, 2, D], F32, tag="x", name="x")
            for b in (0, 1):
                if triv_bias:
                    pre = psB[:, b, :]
                else:
                    t1b = xpool.tile([128, D], F32, tag="t1b", name="t1b")
                    nc.vector.tensor_add(t1b[:], psB[:, b, :], t_bias[:])
                    pre = t1b[:]
                nc.vector.scalar_tensor_tensor(
                    x[:, b, :],
                    pre,
                    0.0,
                    hrs[:, b, :],
                    op0=ALU.max,
                    op1=ALU.add,
                    accum_out=xs_g[:, gcol0 + b : gcol0 + b + 1],
                )
            sq = sqpool.tile([128, 2, D], F32, tag="sq", name="sq")
            for b in (0, 1):
                nc.scalar.activation(
                    sq[:, b, :],
                    x[:, b, :],
                    ACTF.Square,
                    accum_out=ssq_g[:, gcol0 + b : gcol0 + b + 1],
                )
            pending.append((w, x))
            if w % GLN == GLN - 1 or w == nw - 1:
                flush_group(pending)
                pending = []
    nc.compile()
    return nc


def _make_in_maps(h, edge_index, W_self, W_msg, bias, gamma, beta, g, prep):
    B, N, D, NG, n_core, nw, half = _geometry(g)
    T_low, T_high, cl, ch, per_group = prep
    n_pad = nw * 128

    h = np.asarray(h, np.float32)
    # Interleaved dual-batch bf16 node table: row n = [h[0,n] | h[1,n]].
    h2 = np.concatenate([h[0], h[1]], axis=1).astype(NPBF)
    h2lo = np.ascontiguousarray(h2[:half])
    h2hi = np.ascontiguousarray(h2[half:])
    wsT = np.ascontiguousarray(np.asarray(W_self, np.float32).T.astype(NPBF))
    wmT = np.ascontiguousarray(np.asarray(W_msg, np.float32).T.astype(NPBF))
    iota = np.ascontiguousarray(
        np.broadcast_to(np.arange(128, dtype=np.float32)[None, :], (128, 128)).astype(
            NPBF
        )
    )
    ident = np.ascontiguousarray(np.eye(128, dtype=np.float32).astype(NPBF))
    trivial = _trivial_flags(bias, gamma, beta)
    bias_b = np.ascontiguousarray(
        np.broadcast_to(np.asarray(bias, np.float32)[None, :], (128, D))
    )
    gam_b = np.ascontiguousarray(
        np.broadcast_to(np.asarray(gamma, np.float32)[None, :], (128, D))
    )
    bet_b = np.ascontiguousarray(
        np.broadcast_to(np.asarray(beta, np.float32)[None, :], (128, D))
    )

    # Pad-row pattern keeps pad-row LayerNorm variance bounded away from 0.
    padrow = (0.001 * (1.0 - 2.0 * (np.arange(D) % 2))).astype(np.float32)

    in_maps = []
    for c in range(NG):
        pg = per_group[c]
        perm = pg["perm"]
        valid = perm >= 0
        hpad = np.empty((B, n_pad, D), np.float32)
        hpad[:, :] = padrow[None, None, :]
        for b in range(B):
            hpad[b][valid] = h[b][perm[valid]]
        # hT[p=f, w, b, n]
        hp = hpad.reshape(B, nw, 128, D)
        hT = np.ascontiguousarray(hp.transpose(3, 1, 0, 2).astype(NPBF))
        m = {
            "h2lo": h2lo,
            "h2hi": h2hi,
            "ixlo": pg["ixlo"],
            "ixhi": pg["ixhi"],
            "dllo": pg["dllo"],
            "dlhi": pg["dlhi"],
            "crlo": pg["crlo"],
            "crhi": pg["crhi"],
            "hT": hT,
            "wsT": wsT,
            "wmT": wmT,
            "iota": iota,
            "ident": ident,
        }
        if not trivial[0]:
            m["bias_b"] = bias_b
        if not trivial[1]:
            m["gamma_b"] = gam_b
        if not trivial[2]:
            m["beta_b"] = bet_b
        in_maps.append(m)
    return in_maps


def _trivial_flags(bias, gamma, beta):
    return (
        not np.any(np.asarray(bias)),
        bool(np.all(np.asarray(gamma) == 1.0)),
        not np.any(np.asarray(beta)),
    )


def _decode_out(arr, perm, out, nw):
    # arr: [128, nw, 2, D] bf16; row (w*128+p) of batch b = arr[p, w, b, :].
    valid = perm >= 0
    a = np.asarray(arr).transpose(1, 0, 2, 3).reshape(nw * 128, 2, -1)
    af = a[valid].astype(np.float32)
    out[0, perm[valid]] = af[:, 0]
    out[1, perm[valid]] = af[:, 1]


def kernel(h, edge_index, W_self, W_msg, bias, gamma, beta):
    global LAST_RESULTS
    g = FULL_GEO
    B, N, D, NG, n_core, nw, half = _geometry(g)
    prep = _preprocess(edge_index, g)
    T_low, T_high, cl, ch, per_group = prep
    trivial = _trivial_flags(bias, gamma, beta)
    nc = _build_program(g, T_low, T_high, cl, ch, trivial)
    in_maps = _make_in_maps(h, edge_index, W_self, W_msg, bias, gamma, beta, g, prep)
    res = run_bass_kernel_spmd(nc, in_maps, core_ids=list(range(NG)))
    LAST_RESULTS = res
    out = np.empty((B, N, D), np.float32)
    for c in range(NG):
        _decode_out(res.results[c]["out"], per_group[c]["perm"], out, nw)
    return out


# revision 18
# speedup vs baseline: 1.1836x; 1.1836x over previous
"""Trainium2 Bass kernel for MessagePassingLayerV1 (bf16 dual-batch design).

Reference computation (per batch b):
    h_self = h @ W_self.T
    msg    = h[:, src, :] @ W_msg.T               (per edge)
    h_agg[n] = mean over {e: dst[e]==n} of msg[e]  (count clamped >= 1)
    x = h + relu(h_self + h_agg + bias)
    out = LayerNorm(x) * gamma + beta

Key restructures vs the fp32 baseline:
  * W_msg applied AFTER the mean (linearity), so only raw h[src] is gathered.
  * Both batches share edge_index, so each node's features for BOTH batches
    are interleaved into one bf16 row of 512B: ONE dma_gather descriptor per
    edge covers both batches (half the descriptors, half the bytes of the
    fp32 single-batch scheme; 512B is the DMA full-rate boundary).
  * All matmuls bf16: 1 cycle/row vs fp32's 4 (tolerance is 2e-2).
  * Scatter-add via matmul: per 128-edge tile, aggT[f,dst] += X.T @ S with
    S[e,j] = (iota[j] == slot[e]) * (1/count[dst[e]]) built per tile by one
    DVE tensor_scalar (is_equal, mult) — bf16 in/out with f32 per-partition
    scalars keeps the 4x_2p DVE fast path. aggT copies PSUM->SBUF on the ACT
    engine (Copy, bf16 out).
  * h rows for the residual are NOT loaded: they are recovered on-chip by
    PE-transposing the (needed anyway) hT tiles into PSUM; the relu+residual
    DVE op reads them straight from PSUM.
  * LayerNorm stats for 4 windows x 2 batches are packed into [128,8] tiles
    so the small-op chain runs once per 4 windows; eps dropped (pad rows get
    a +-1e-3 pattern so var >= ~1e-6; relative effect < 1e-5 on real rows);
    y emitted bf16 on ACT via Identity(x*rstd - mu*rstd).

Sharding: 8 cores x (1/8 of dst nodes, BOTH batches). Single SPMD program:
per-(window, half) tile counts are padded to the max across the 8 groups.
No collectives; host assembles the 8 disjoint output shards.
"""

import sys
from contextlib import ExitStack

import numpy as np

sys.path.insert(0, "/opt/trn_rl_repo")

import ml_dtypes  # noqa: E402

import concourse.bacc as bacc  # noqa: E402
import concourse.bass as bass  # noqa: E402
import concourse.mybir as mybir  # noqa: E402
import concourse.tile as tile  # noqa: E402
from concourse._compat import get_trn_type as _get_trn_type  # noqa: E402
from concourse.bass_utils import run_bass_kernel_spmd  # noqa: E402
from concourse.library_config import mlp as _mlp_library  # noqa: E402

F32 = mybir.dt.float32
BF16 = mybir.dt.bfloat16
I16 = mybir.dt.int16
ALU = mybir.AluOpType
ACTF = mybir.ActivationFunctionType
NPBF = ml_dtypes.bfloat16

PAD_DLOC = 200.0  # dst_local sentinel: never equals iota 0..127 -> S row = 0

# Full-problem geometry (hardcoded per harness contract).
FULL_GEO = dict(B=2, N=50000, D=128, NG=8, NW_EXTRA=1)

GMAX = 8  # gather tiles (128 idx each) per dma_gather call (1024-idx ucode cap)
SCRATCH = 65536  # SWDGE descriptor ring: 4096 descs = 4 calls in flight
CHW = 10  # hT chunk size in windows
GLN = 4  # windows per LayerNorm stats batch

# Holder for the last run's BassKernelResults (test.py reads exec_time_ns).
LAST_RESULTS = None


def _geometry(g):
    B, N, NG = g["B"], g["N"], g["NG"]
    n_core = N // NG
    assert n_core * NG == N
    nw = -(-n_core // 128) + g.get("NW_EXTRA", 0)
    half = N // 2
    return B, N, g["D"], NG, n_core, nw, half


def _preprocess(edge_index, g):
    """Per-group edge metadata, padded to uniform tile counts across groups.

    Nodes are assigned to 128-slot windows with a degree-balanced greedy so
    per-window-half edge counts are nearly equal across windows AND groups.
    Returns (T_low, T_high, cl, ch, per_group); per_group[q] has ixlo/ixhi
    (wrapped int16), dllo/dlhi + crlo/crhi (f32 [128,T]), perm ([nw*128]
    global node id per slot, -1 = pad).
    """
    B, N, D, NG, n_core, nw, half = _geometry(g)
    n_pad = nw * 128
    src = np.asarray(edge_index[0]).astype(np.int64)
    dst = np.asarray(edge_index[1]).astype(np.int64)
    counts = np.bincount(dst, minlength=N).astype(np.float32)
    crec_node = (1.0 / np.maximum(counts, 1.0)).astype(np.float32)

    groups = {}
    nlow = np.zeros((NG, nw), np.int64)
    nhigh = np.zeros((NG, nw), np.int64)
    perms = []
    lo_edge = src < half
    degs = []
    for q in range(NG):
        base = q * n_core
        qsel = (dst >= base) & (dst < base + n_core)
        dloc_all = dst[qsel] - base
        deg_lo = np.bincount(dloc_all[lo_edge[qsel]], minlength=n_core)
        deg_hi = np.bincount(dloc_all[~lo_edge[qsel]], minlength=n_core)
        degs.append((deg_lo, deg_hi))
    base_tiles = max(
        1,
        int(np.ceil(max(max(dl.sum(), dh.sum()) for dl, dh in degs) / nw / 128)),
    )
    cap0 = 128 * base_tiles
    nspill = [
        int(np.ceil(max(0.0, max(dl.sum(), dh.sum()) - cap0 * nw) / 128))
        for dl, dh in degs
    ]
    nspill_max = max(nspill)
    caps = np.full(nw, cap0)
    caps[:nspill_max] = cap0 + 128
    for q in range(NG):
        base = q * n_core
        deg_lo, deg_hi = degs[q]
        order = np.argsort(-(deg_lo + deg_hi), kind="stable")
        n_lo = np.zeros(nw)
        n_hi = np.zeros(nw)
        fill = np.zeros(nw, np.int64)
        wof = np.empty(n_core, np.int64)
        slot = np.empty(n_core, np.int64)
        perm = np.full(n_pad, -1, np.int64)
        tcap = caps / 128.0
        for nl in order:
            a = n_lo + deg_lo[nl]
            b = n_hi + deg_hi[nl]
            pen = (
                np.maximum(np.ceil(a / 128.0) - tcap, 0)
                - np.maximum(np.ceil(n_lo / 128.0) - tcap, 0)
                + np.maximum(np.ceil(b / 128.0) - tcap, 0)
                - np.maximum(np.ceil(n_hi / 128.0) - tcap, 0)
            )
            score = np.maximum(a, b) + 1e6 * pen
            score[fill >= 128] = np.inf
            w = int(np.argmin(score))
            wof[nl] = w
            slot[nl] = fill[w]
            perm[w * 128 + fill[w]] = base + nl
            fill[w] += 1
            n_lo[w] += deg_lo[nl]
            n_hi[w] += deg_hi[nl]
        assert fill.max() <= 128
        perms.append(perm)

        sel = (dst >= base) & (dst < base + n_core)
        s_q = src[sel]
        d_loc = dst[sel] - base
        w_e = wof[d_loc]
        o1 = np.lexsort((s_q, w_e))
        s_q, d_loc, w_e = s_q[o1], d_loc[o1], w_e[o1]
        bounds = np.searchsorted(w_e, np.arange(nw + 1))
        for w in range(nw):
            sw = s_q[bounds[w] : bounds[w + 1]]
            dw = d_loc[bounds[w] : bounds[w + 1]]
            lo = sw < half
            for tag, mask, sbase in (("lo", lo, 0), ("hi", ~lo, half)):
                s_g = sw[mask] - sbase
                d_g = dw[mask]
                o2 = np.argsort(s_g, kind="stable")
                groups[(q, w, tag)] = (
                    s_g[o2],
                    slot[d_g[o2]].astype(np.float32),
                    crec_node[d_g[o2] + base],
                )
                if tag == "lo":
                    nlow[q, w] = s_g.size
                else:
                    nhigh[q, w] = s_g.size

    T_low = -(-nlow.max(axis=0) // 128)
    T_high = -(-nhigh.max(axis=0) // 128)
    empty = (T_low + T_high) == 0
    T_low[empty] = 1
    cl = np.concatenate([[0], np.cumsum(T_low)]).astype(np.int64)
    ch = np.concatenate([[0], np.cumsum(T_high)]).astype(np.int64)

    def wrap_idx(arr):
        # dma_gather layout: idx j -> partition j%16, col j//16; replicated x8.
        a = arr.reshape(-1, 16).T.astype(np.int16)
        return np.ascontiguousarray(np.tile(a, (8, 1)))

    per_group = []
    for q in range(NG):
        out = {}
        for tag, T, cum in (("lo", T_low, cl), ("hi", T_high, ch)):
            tot = int(cum[-1])
            idx = np.zeros(tot * 128, np.int64)
            dl = np.full(tot * 128, PAD_DLOC, np.float32)
            cr = np.zeros(tot * 128, np.float32)
            for w in range(nw):
                s_g, d_g, c_g = groups[(q, w, tag)]
                off = int(cum[w]) * 128
                idx[off : off + s_g.size] = s_g
                dl[off : off + s_g.size] = d_g
                cr[off : off + s_g.size] = c_g
            out["ix" + tag] = wrap_idx(idx)
            out["dl" + tag] = np.ascontiguousarray(dl.reshape(tot, 128).T)
            out["cr" + tag] = np.ascontiguousarray(cr.reshape(tot, 128).T)
        out["perm"] = perms[q]
        per_group.append(out)
    return T_low, T_high, cl, ch, per_group


def _build_program(g, T_low, T_high, cl, ch, trivial=(True, True, True)):
    B, N, D, NG, n_core, nw, half = _geometry(g)
    TL, TH = int(cl[-1]), int(ch[-1])
    triv_bias, triv_gamma, triv_beta = trivial
    D2 = 2 * D

    nc = bacc.Bacc(
        _get_trn_type() or "TRN2",
        target_bir_lowering=False,
        debug=False,
        num_devices=NG,
        dynamic_dma_scratch_size=SCRATCH,
    )
    d_hlo = nc.dram_tensor("h2lo", [half, D2], BF16, kind="ExternalInput")
    d_hhi = nc.dram_tensor("h2hi", [N - half, D2], BF16, kind="ExternalInput")
    d_ixlo = nc.dram_tensor("ixlo", [128, TL * 8], I16, kind="ExternalInput")
    d_ixhi = nc.dram_tensor("ixhi", [128, TH * 8], I16, kind="ExternalInput")
    d_dllo = nc.dram_tensor("dllo", [128, TL], F32, kind="ExternalInput")
    d_dlhi = nc.dram_tensor("dlhi", [128, TH], F32, kind="ExternalInput")
    d_crlo = nc.dram_tensor("crlo", [128, TL], F32, kind="ExternalInput")
    d_crhi = nc.dram_tensor("crhi", [128, TH], F32, kind="ExternalInput")
    d_hT = nc.dram_tensor("hT", [128, nw, 2, D], BF16, kind="ExternalInput")
    d_wsT = nc.dram_tensor("wsT", [D, D], BF16, kind="ExternalInput")
    d_wmT = nc.dram_tensor("wmT", [D, D], BF16, kind="ExternalInput")
    d_iota = nc.dram_tensor("iota", [128, 128], BF16, kind="ExternalInput")
    d_ident = nc.dram_tensor("ident", [128, 128], BF16, kind="ExternalInput")
    d_bias = d_gam = d_bet = None
    if not triv_bias:
        d_bias = nc.dram_tensor("bias_b", [128, D], F32, kind="ExternalInput")
    if not triv_gamma:
        d_gam = nc.dram_tensor("gamma_b", [128, D], F32, kind="ExternalInput")
    if not triv_beta:
        d_bet = nc.dram_tensor("beta_b", [128, D], F32, kind="ExternalInput")
    d_out = nc.dram_tensor("out", [128, nw, 2, D], BF16, kind="ExternalOutput")

    with tile.TileContext(nc) as tc, ExitStack() as ctx:
        cpool = ctx.enter_context(tc.tile_pool(name="const", bufs=1))
        gplo = ctx.enter_context(tc.tile_pool(name="glo", bufs=4))
        gphi = ctx.enter_context(tc.tile_pool(name="ghi", bufs=4))
        htp = ctx.enter_context(tc.tile_pool(name="htp", bufs=2))
        spool = ctx.enter_context(tc.tile_pool(name="sel", bufs=8))
        apool = ctx.enter_context(tc.tile_pool(name="aggts", bufs=4))
        xpool = ctx.enter_context(tc.tile_pool(name="xp", bufs=GLN + 2))
        sqpool = ctx.enter_context(tc.tile_pool(name="sqp", bufs=2))
        ypool = ctx.enter_context(tc.tile_pool(name="yp", bufs=6))
        stpool = ctx.enter_context(tc.tile_pool(name="stats", bufs=2))
        ppA = ctx.enter_context(
            tc.tile_pool(name="psA", bufs=3, space=bass.MemorySpace.PSUM)
        )
        ppB = ctx.enter_context(
            tc.tile_pool(name="psB", bufs=3, space=bass.MemorySpace.PSUM)
        )
        ppT = ctx.enter_context(
            tc.tile_pool(name="psT", bufs=2, space=bass.MemorySpace.PSUM)
        )

        nc.gpsimd.load_library(_mlp_library)

        def cload(dram, shape, dtype=BF16):
            t = cpool.tile(shape, dtype, tag=dram.name, name=dram.name + "_t")
            nc.sync.dma_start(t[:], dram[:])
            return t

        t_ixlo = cload(d_ixlo, [128, TL * 8], I16)
        t_ixhi = cload(d_ixhi, [128, TH * 8], I16)
        t_dllo = cload(d_dllo, [128, TL], F32)
        t_dlhi = cload(d_dlhi, [128, TH], F32)
        t_crlo = cload(d_crlo, [128, TL], F32)
        t_crhi = cload(d_crhi, [128, TH], F32)
        t_iota = cload(d_iota, [128, 128])
        t_ident = cload(d_ident, [128, 128])
        t_wsT = cload(d_wsT, [D, D])
        t_wmT = cload(d_wmT, [D, D])
        t_bias = None if triv_bias else cload(d_bias, [128, D], F32)
        t_gam = None if triv_gamma else cload(d_gam, [128, D], F32)
        t_bet = None if triv_beta else cload(d_bet, [128, D], F32)

        blocks = {"lo": [], "hi": []}
        issued = {"lo": 0, "hi": 0}
        totals = {"lo": TL, "hi": TH}
        gsrc = {"lo": d_hlo, "hi": d_hhi}
        gix = {"lo": t_ixlo, "hi": t_ixhi}
        gpool = {"lo": gplo, "hi": gphi}

        def ensure_gathered(kind, upto):
            while issued[kind] < min(upto, totals[kind]):
                t0 = issued[kind]
                t1 = min(t0 + GMAX, totals[kind])
                blk = gpool[kind].tile(
                    [128, GMAX, D2], BF16, tag="g" + kind, name="g" + kind
                )
                nc.gpsimd.dma_gather(
                    blk[:, 0 : t1 - t0, :],
                    gsrc[kind][:],
                    gix[kind][:, t0 * 8 : t1 * 8],
                    num_idxs=(t1 - t0) * 128,
                    num_idxs_reg=(t1 - t0) * 128,
                    elem_size=D2,
                )
                blocks[kind].append(blk)
                issued[kind] = t1

        ht_tiles = {}
        pending = []  # (w, x_tile) awaiting group LayerNorm stats
        xs_g = ssq_g = None

        def flush_group(wlist):
            nonlocal xs_g, ssq_g
            gw = len(wlist)
            mu = stpool.tile([128, 2 * GLN], F32, tag="mu", name="mu")
            nc.vector.tensor_scalar_mul(mu[:, 0 : 2 * gw], xs_g[:, 0 : 2 * gw], 1.0 / D)
            nmusq = stpool.tile([128, 2 * GLN], F32, tag="nmusq", name="nmusq")
            nc.vector.scalar_tensor_tensor(
                nmusq[:, 0 : 2 * gw],
                mu[:, 0 : 2 * gw],
                -1.0,
                mu[:, 0 : 2 * gw],
                op0=ALU.mult,
                op1=ALU.mult,
            )
            varv = stpool.tile([128, 2 * GLN], F32, tag="varv", name="varv")
            nc.vector.scalar_tensor_tensor(
                varv[:, 0 : 2 * gw],
                ssq_g[:, 0 : 2 * gw],
                1.0 / D,
                nmusq[:, 0 : 2 * gw],
                op0=ALU.mult,
                op1=ALU.add,
            )
            vrec = stpool.tile([128, 2 * GLN], F32, tag="vrec", name="vrec")
            nc.vector.reciprocal(vrec[:, 0 : 2 * gw], varv[:, 0 : 2 * gw])
            rstd = stpool.tile([128, 2 * GLN], F32, tag="rstd", name="rstd")
            nc.scalar.activation(rstd[:, 0 : 2 * gw], vrec[:, 0 : 2 * gw], ACTF.Sqrt)
            nmr = stpool.tile([128, 2 * GLN], F32, tag="nmr", name="nmr")
            nc.vector.scalar_tensor_tensor(
                nmr[:, 0 : 2 * gw],
                mu[:, 0 : 2 * gw],
                -1.0,
                rstd[:, 0 : 2 * gw],
                op0=ALU.mult,
                op1=ALU.mult,
            )
            for wi, (w2, x2) in enumerate(wlist):
                y = ypool.tile([128, 2, D], BF16, tag="y", name="y")
                for b in (0, 1):
                    c = wi * 2 + b
                    yt = y[:, b, :]
                    if not (triv_gamma and triv_beta):
                        yf = ypool.tile([128, D], F32, tag="yf", name="yf")
                        yt = yf[:]
                    nc.scalar.activation(
                        yt,
                        x2[:, b, :],
                        ACTF.Identity,
                        bias=nmr[:, c : c + 1],
                        scale=rstd[:, c : c + 1],
                    )
                    if not triv_gamma:
                        yg = ypool.tile([128, D], F32, tag="yg", name="yg")
                        nc.vector.tensor_mul(yg[:], yt, t_gam[:])
                        yt = yg[:]
                    if not triv_beta:
                        nc.vector.tensor_add(y[:, b, :], yt, t_bet[:])
                    elif not triv_gamma:
                        nc.vector.tensor_copy(y[:, b, :], yt)
                nc.sync.dma_start(d_out[:, w2, :, :], y[:])

        for w in range(nw):
            tlc = int(T_low[w])
            thc = int(T_high[w])
            cl0, ch0 = int(cl[w]), int(ch[w])
            ensure_gathered("lo", cl0 + tlc)
            ensure_gathered("hi", ch0 + thc)

            ci = w // CHW
            want = [ci, ci + 1] if w % CHW >= CHW - 3 else [ci]
            for cli in want:
                if cli not in ht_tiles and cli * CHW < nw:
                    c0 = cli * CHW
                    c1 = min(c0 + CHW, nw)
                    tht2 = htp.tile([128, CHW, 2, D], BF16, tag="ht", name="ht")
                    nc.sync.dma_start(
                        tht2[:, 0 : c1 - c0, :, :], d_hT[:, c0:c1, :, :]
                    )
                    ht_tiles[cli] = tht2
            tht = ht_tiles[ci]
            k = w % CHW

            if w % GLN == 0:
                xs_g = stpool.tile([128, 2 * GLN], F32, tag="xs", name="xs")
                ssq_g = stpool.tile([128, 2 * GLN], F32, tag="ssq", name="ssq")
            gcol0 = 2 * (w % GLN)

            # S tiles: one-hot * 1/count, one fused DVE tensor_scalar per tile
            # (bf16 in/out + per-partition f32 scalars -> 4x DVE mode).
            ntile = tlc + thc
            S = spool.tile([128, ntile, 128], BF16, tag="S", name="S")
            tiles = [("lo", cl0 + kk, kk, t_dllo, t_crlo) for kk in range(tlc)] + [
                ("hi", ch0 + kk, tlc + kk, t_dlhi, t_crhi) for kk in range(thc)
            ]
            for kind, t, scol, t_dl, t_cr in tiles:
                nc.vector.tensor_scalar(
                    S[:, scol, :],
                    t_iota[:],
                    t_dl[:, t : t + 1],
                    t_cr[:, t : t + 1],
                    op0=ALU.is_equal,
                    op1=ALU.mult,
                )

            # aggT[f, dst] per batch, accumulated over edge tiles in PSUM.
            # Batch groups run sequentially so both fit one PSUM bank.
            psA = ppA.tile([128, 2, 128], F32, tag="psA", name="psA")
            for b in (0, 1):
                for j, (kind, t, scol, _dl, _cr) in enumerate(tiles):
                    blk = blocks[kind][t // GMAX]
                    slot = t % GMAX
                    nc.tensor.matmul(
                        psA[:, b, :],
                        blk[:, slot, b * D : (b + 1) * D],
                        S[:, scol, :],
                        start=j == 0,
                        stop=j == len(tiles) - 1,
                    )

            # PSUM -> SBUF bf16 copy of aggT on the (otherwise idle) ACT engine.
            aggTs = apool.tile([128, 2, 128], BF16, tag="aggTs", name="aggTs")
            nc.vector.tensor_copy(aggTs[:], psA[:])

            # h rows (residual) recovered on-chip: hrowT = transpose(hT).
            # Vector ops may read only ONE PSUM input (psB takes that slot),
            # so stage the transposed rows to SBUF, one copy per engine.
            hrT = ppT.tile([128, 2, D], BF16, tag="hrT", name="hrT")
            for b in (0, 1):
                nc.tensor.transpose(hrT[:, b, :], tht[:, k, b, :], t_ident[:])
            hrs = apool.tile([128, 2, D], BF16, tag="hrs", name="hrs")
            nc.scalar.activation(hrs[:], hrT[:], ACTF.Copy)

            # psB[node, fo] = h @ W_self.T + h_agg @ W_msg.T  (per batch)
            psB = ppB.tile([128, 2, 128], F32, tag="psB", name="psB")
            for b in (0, 1):
                nc.tensor.matmul(
                    psB[:, b, :], tht[:, k, b, :], t_wsT[:], start=True, stop=False
                )
                nc.tensor.matmul(
                    psB[:, b, :], aggTs[:, b, :], t_wmT[:], start=False, stop=True
                )

            # x = h + relu(psB [+ bias]); row-sum accum into the group tile.
            x = xpool.tile([128, 2, D], F32, tag="x", name="x")
            for b in (0, 1):
                if triv_bias:
                    pre = psB[:, b, :]
                else:
                    t1b = xpool.tile([128, D], F32, tag="t1b", name="t1b")
                    nc.vector.tensor_add(t1b[:], psB[:, b, :], t_bias[:])
                    pre = t1b[:]
                nc.vector.scalar_tensor_tensor(
                    x[:, b, :],
                    pre,
                    0.0,
                    hrs[:, b, :],
                    op0=ALU.max,
                    op1=ALU.add,
                    accum_out=xs_g[:, gcol0 + b : gcol0 + b + 1],
                )
            sq = sqpool.tile([128, 2, D], F32, tag="sq", name="sq")
            for b in (0, 1):
                nc.scalar.activation(
                    sq[:, b, :],
                    x[:, b, :],
                    ACTF.Square,
                    accum_out=ssq_g[:, gcol0 + b : gcol0 + b + 1],
                )
            pending.append((w, x))
            if w % GLN == GLN - 1 or w == nw - 1:
                flush_group(pending)
                pending = []
    nc.compile()
    return nc


def _make_in_maps(h, edge_index, W_self, W_msg, bias, gamma, beta, g, prep):
    B, N, D, NG, n_core, nw, half = _geometry(g)
    T_low, T_high, cl, ch, per_group = prep
    n_pad = nw * 128

    h = np.asarray(h, np.float32)
    # Interleaved dual-batch bf16 node table: row n = [h[0,n] | h[1,n]].
    h2 = np.concatenate([h[0], h[1]], axis=1).astype(NPBF)
    h2lo = np.ascontiguousarray(h2[:half])
    h2hi = np.ascontiguousarray(h2[half:])
    wsT = np.ascontiguousarray(np.asarray(W_self, np.float32).T.astype(NPBF))
    wmT = np.ascontiguousarray(np.asarray(W_msg, np.float32).T.astype(NPBF))
    iota = np.ascontiguousarray(
        np.broadcast_to(np.arange(128, dtype=np.float32)[None, :], (128, 128)).astype(
            NPBF
        )
    )
    ident = np.ascontiguousarray(np.eye(128, dtype=np.float32).astype(NPBF))
    trivial = _trivial_flags(bias, gamma, beta)
    bias_b = np.ascontiguousarray(
        np.broadcast_to(np.asarray(bias, np.float32)[None, :], (128, D))
    )
    gam_b = np.ascontiguousarray(
        np.broadcast_to(np.asarray(gamma, np.float32)[None, :], (128, D))
    )
    bet_b = np.ascontiguousarray(
        np.broadcast_to(np.asarray(beta, np.float32)[None, :], (128, D))
    )

    # Pad-row pattern keeps pad-row LayerNorm variance bounded away from 0.
    padrow = (0.001 * (1.0 - 2.0 * (np.arange(D) % 2))).astype(np.float32)

    in_maps = []
    for c in range(NG):
        pg = per_group[c]
        perm = pg["perm"]
        valid = perm >= 0
        hpad = np.empty((B, n_pad, D), np.float32)
        hpad[:, :] = padrow[None, None, :]
        for b in range(B):
            hpad[b][valid] = h[b][perm[valid]]
        # hT[p=f, w, b, n]
        hp = hpad.reshape(B, nw, 128, D)
        hT = np.ascontiguousarray(hp.transpose(3, 1, 0, 2).astype(NPBF))
        m = {
            "h2lo": h2lo,
            "h2hi": h2hi,
            "ixlo": pg["ixlo"],
            "ixhi": pg["ixhi"],
            "dllo": pg["dllo"],
            "dlhi": pg["dlhi"],
            "crlo": pg["crlo"],
            "crhi": pg["crhi"],
            "hT": hT,
            "wsT": wsT,
            "wmT": wmT,
            "iota": iota,
            "ident": ident,
        }
        if not trivial[0]:
            m["bias_b"] = bias_b
        if not trivial[1]:
            m["gamma_b"] = gam_b
        if not trivial[2]:
            m["beta_b"] = bet_b
        in_maps.append(m)
    return in_maps


def _trivial_flags(bias, gamma, beta):
    return (
        not np.any(np.asarray(bias)),
        bool(np.all(np.asarray(gamma) == 1.0)),
        not np.any(np.asarray(beta)),
    )


def _decode_out(arr, perm, out, nw):
    # arr: [128, nw, 2, D] bf16; row (w*128+p) of batch b = arr[p, w, b, :].
    valid = perm >= 0
    a = np.asarray(arr).transpose(1, 0, 2, 3).reshape(nw * 128, 2, -1)
    af = a[valid].astype(np.float32)
    out[0, perm[valid]] = af[:, 0]
    out[1, perm[valid]] = af[:, 1]


def kernel(h, edge_index, W_self, W_msg, bias, gamma, beta):
    global LAST_RESULTS
    g = FULL_GEO
    B, N, D, NG, n_core, nw, half = _geometry(g)
    prep = _preprocess(edge_index, g)
    T_low, T_high, cl, ch, per_group = prep
    trivial = _trivial_flags(bias, gamma, beta)
    nc = _build_program(g, T_low, T_high, cl, ch, trivial)
    in_maps = _make_in_maps(h, edge_index, W_self, W_msg, bias, gamma, beta, g, prep)
    res = run_bass_kernel_spmd(nc, in_maps, core_ids=list(range(NG)))
    LAST_RESULTS = res
    out = np.empty((B, N, D), np.float32)
    for c in range(NG):
        _decode_out(res.results[c]["out"], per_group[c]["perm"], out, nw)
    return out


# revision 19
# speedup vs baseline: 1.1843x; 1.0006x over previous
"""Trainium2 Bass kernel for MessagePassingLayerV1 (bf16 dual-batch design).

Reference computation (per batch b):
    h_self = h @ W_self.T
    msg    = h[:, src, :] @ W_msg.T               (per edge)
    h_agg[n] = mean over {e: dst[e]==n} of msg[e]  (count clamped >= 1)
    x = h + relu(h_self + h_agg + bias)
    out = LayerNorm(x) * gamma + beta

Key restructures vs the fp32 baseline:
  * W_msg applied AFTER the mean (linearity), so only raw h[src] is gathered.
  * Both batches share edge_index, so each node's features for BOTH batches
    are interleaved into one bf16 row of 512B: ONE dma_gather descriptor per
    edge covers both batches (half the descriptors, half the bytes of the
    fp32 single-batch scheme; 512B is the DMA full-rate boundary).
  * All matmuls bf16: 1 cycle/row vs fp32's 4 (tolerance is 2e-2).
  * Scatter-add via matmul: per 128-edge tile, aggT[f,dst] += X.T @ S with
    S[e,j] = (iota[j] == slot[e]) * (1/count[dst[e]]) built per tile by one
    DVE tensor_scalar (is_equal, mult) — bf16 in/out with f32 per-partition
    scalars keeps the 4x_2p DVE fast path. aggT copies PSUM->SBUF on the ACT
    engine (Copy, bf16 out).
  * h rows for the residual are NOT loaded: they are recovered on-chip by
    PE-transposing the (needed anyway) hT tiles into PSUM; the relu+residual
    DVE op reads them straight from PSUM.
  * LayerNorm stats for 4 windows x 2 batches are packed into [128,8] tiles
    so the small-op chain runs once per 4 windows; eps dropped (pad rows get
    a +-1e-3 pattern so var >= ~1e-6; relative effect < 1e-5 on real rows);
    y emitted bf16 on ACT via Identity(x*rstd - mu*rstd).

Sharding: 8 cores x (1/8 of dst nodes, BOTH batches). Single SPMD program:
per-(window, half) tile counts are padded to the max across the 8 groups.
No collectives; host assembles the 8 disjoint output shards.
"""

import sys
from contextlib import ExitStack

import numpy as np

sys.path.insert(0, "/opt/trn_rl_repo")

import ml_dtypes  # noqa: E402

import concourse.bacc as bacc  # noqa: E402
import concourse.bass as bass  # noqa: E402
import concourse.mybir as mybir  # noqa: E402
import concourse.tile as tile  # noqa: E402
from concourse._compat import get_trn_type as _get_trn_type  # noqa: E402
from concourse.bass_utils import run_bass_kernel_spmd  # noqa: E402
from concourse.library_config import mlp as _mlp_library  # noqa: E402

F32 = mybir.dt.float32
BF16 = mybir.dt.bfloat16
I16 = mybir.dt.int16
ALU = mybir.AluOpType
ACTF = mybir.ActivationFunctionType
NPBF = ml_dtypes.bfloat16

PAD_DLOC = 200.0  # dst_local sentinel: never equals iota 0..127 -> S row = 0

# Full-problem geometry (hardcoded per harness contract).
FULL_GEO = dict(B=2, N=50000, D=128, NG=8, NW_EXTRA=1)

GMAX = 8  # gather tiles (128 idx each) per dma_gather call (1024-idx ucode cap)
SCRATCH = 65536  # SWDGE descriptor ring: 4096 descs = 4 calls in flight
CHW = 10  # hT chunk size in windows
GLN = 4  # windows per LayerNorm stats batch

# Holder for the last run's BassKernelResults (test.py reads exec_time_ns).
LAST_RESULTS = None


def _geometry(g):
    B, N, NG = g["B"], g["N"], g["NG"]
    n_core = N // NG
    assert n_core * NG == N
    nw = -(-n_core // 128) + g.get("NW_EXTRA", 0)
    half = N // 2
    return B, N, g["D"], NG, n_core, nw, half


def _preprocess(edge_index, g):
    """Per-group edge metadata, padded to uniform tile counts across groups.

    Nodes are assigned to 128-slot windows with a degree-balanced greedy so
    per-window-half edge counts are nearly equal across windows AND groups.
    Returns (T_low, T_high, cl, ch, per_group); per_group[q] has ixlo/ixhi
    (wrapped int16), dllo/dlhi + crlo/crhi (f32 [128,T]), perm ([nw*128]
    global node id per slot, -1 = pad).
    """
    B, N, D, NG, n_core, nw, half = _geometry(g)
    n_pad = nw * 128
    src = np.asarray(edge_index[0]).astype(np.int64)
    dst = np.asarray(edge_index[1]).astype(np.int64)
    counts = np.bincount(dst, minlength=N).astype(np.float32)
    crec_node = (1.0 / np.maximum(counts, 1.0)).astype(np.float32)

    groups = {}
    nlow = np.zeros((NG, nw), np.int64)
    nhigh = np.zeros((NG, nw), np.int64)
    perms = []
    lo_edge = src < half
    degs = []
    for q in range(NG):
        base = q * n_core
        qsel = (dst >= base) & (dst < base + n_core)
        dloc_all = dst[qsel] - base
        deg_lo = np.bincount(dloc_all[lo_edge[qsel]], minlength=n_core)
        deg_hi = np.bincount(dloc_all[~lo_edge[qsel]], minlength=n_core)
        degs.append((deg_lo, deg_hi))
    base_tiles = max(
        1,
        int(np.ceil(max(max(dl.sum(), dh.sum()) for dl, dh in degs) / nw / 128)),
    )
    cap0 = 128 * base_tiles
    nspill = [
        int(np.ceil(max(0.0, max(dl.sum(), dh.sum()) - cap0 * nw) / 128))
        for dl, dh in degs
    ]
    nspill_max = max(nspill)
    caps = np.full(nw, cap0)
    caps[:nspill_max] = cap0 + 128
    for q in range(NG):
        base = q * n_core
        deg_lo, deg_hi = degs[q]
        order = np.argsort(-(deg_lo + deg_hi), kind="stable")
        n_lo = np.zeros(nw)
        n_hi = np.zeros(nw)
        fill = np.zeros(nw, np.int64)
        wof = np.empty(n_core, np.int64)
        slot = np.empty(n_core, np.int64)
        perm = np.full(n_pad, -1, np.int64)
        tcap = caps / 128.0
        for nl in order:
            a = n_lo + deg_lo[nl]
            b = n_hi + deg_hi[nl]
            pen = (
                np.maximum(np.ceil(a / 128.0) - tcap, 0)
                - np.maximum(np.ceil(n_lo / 128.0) - tcap, 0)
                + np.maximum(np.ceil(b / 128.0) - tcap, 0)
                - np.maximum(np.ceil(n_hi / 128.0) - tcap, 0)
            )
            score = np.maximum(a, b) + 1e6 * pen
            score[fill >= 128] = np.inf
            w = int(np.argmin(score))
            wof[nl] = w
            slot[nl] = fill[w]
            perm[w * 128 + fill[w]] = base + nl
            fill[w] += 1
            n_lo[w] += deg_lo[nl]
            n_hi[w] += deg_hi[nl]
        assert fill.max() <= 128
        perms.append(perm)

        sel = (dst >= base) & (dst < base + n_core)
        s_q = src[sel]
        d_loc = dst[sel] - base
        w_e = wof[d_loc]
        o1 = np.lexsort((s_q, w_e))
        s_q, d_loc, w_e = s_q[o1], d_loc[o1], w_e[o1]
        bounds = np.searchsorted(w_e, np.arange(nw + 1))
        for w in range(nw):
            sw = s_q[bounds[w] : bounds[w + 1]]
            dw = d_loc[bounds[w] : bounds[w + 1]]
            lo = sw < half
            for tag, mask, sbase in (("lo", lo, 0), ("hi", ~lo, half)):
                s_g = sw[mask] - sbase
                d_g = dw[mask]
                o2 = np.argsort(s_g, kind="stable")
                groups[(q, w, tag)] = (
                    s_g[o2],
                    slot[d_g[o2]].astype(np.float32),
                    crec_node[d_g[o2] + base],
                )
                if tag == "lo":
                    nlow[q, w] = s_g.size
                else:
                    nhigh[q, w] = s_g.size

    T_low = -(-nlow.max(axis=0) // 128)
    T_high = -(-nhigh.max(axis=0) // 128)
    empty = (T_low + T_high) == 0
    T_low[empty] = 1
    cl = np.concatenate([[0], np.cumsum(T_low)]).astype(np.int64)
    ch = np.concatenate([[0], np.cumsum(T_high)]).astype(np.int64)

    def wrap_idx(arr):
        # dma_gather layout: idx j -> partition j%16, col j//16; replicated x8.
        a = arr.reshape(-1, 16).T.astype(np.int16)
        return np.ascontiguousarray(np.tile(a, (8, 1)))

    per_group = []
    for q in range(NG):
        out = {}
        for tag, T, cum in (("lo", T_low, cl), ("hi", T_high, ch)):
            tot = int(cum[-1])
            idx = np.zeros(tot * 128, np.int64)
            dl = np.full(tot * 128, PAD_DLOC, np.float32)
            cr = np.zeros(tot * 128, np.float32)
            for w in range(nw):
                s_g, d_g, c_g = groups[(q, w, tag)]
                off = int(cum[w]) * 128
                idx[off : off + s_g.size] = s_g
                dl[off : off + s_g.size] = d_g
                cr[off : off + s_g.size] = c_g
            out["ix" + tag] = wrap_idx(idx)
            out["dl" + tag] = np.ascontiguousarray(dl.reshape(tot, 128).T)
            out["cr" + tag] = np.ascontiguousarray(cr.reshape(tot, 128).T)
        out["perm"] = perms[q]
        per_group.append(out)
    return T_low, T_high, cl, ch, per_group


def _build_program(g, T_low, T_high, cl, ch, trivial=(True, True, True)):
    B, N, D, NG, n_core, nw, half = _geometry(g)
    TL, TH = int(cl[-1]), int(ch[-1])
    triv_bias, triv_gamma, triv_beta = trivial
    D2 = 2 * D

    nc = bacc.Bacc(
        _get_trn_type() or "TRN2",
        target_bir_lowering=False,
        debug=False,
        num_devices=NG,
        dynamic_dma_scratch_size=SCRATCH,
    )
    d_hlo = nc.dram_tensor("h2lo", [half, D2], BF16, kind="ExternalInput")
    d_hhi = nc.dram_tensor("h2hi", [N - half, D2], BF16, kind="ExternalInput")
    d_ixlo = nc.dram_tensor("ixlo", [128, TL * 8], I16, kind="ExternalInput")
    d_ixhi = nc.dram_tensor("ixhi", [128, TH * 8], I16, kind="ExternalInput")
    d_dllo = nc.dram_tensor("dllo", [128, TL], F32, kind="ExternalInput")
    d_dlhi = nc.dram_tensor("dlhi", [128, TH], F32, kind="ExternalInput")
    d_crlo = nc.dram_tensor("crlo", [128, TL], F32, kind="ExternalInput")
    d_crhi = nc.dram_tensor("crhi", [128, TH], F32, kind="ExternalInput")
    d_hT = nc.dram_tensor("hT", [128, nw, 2, D], BF16, kind="ExternalInput")
    d_wsT = nc.dram_tensor("wsT", [D, D], BF16, kind="ExternalInput")
    d_wmT = nc.dram_tensor("wmT", [D, D], BF16, kind="ExternalInput")
    d_iota = nc.dram_tensor("iota", [128, 128], BF16, kind="ExternalInput")
    d_ident = nc.dram_tensor("ident", [128, 128], BF16, kind="ExternalInput")
    d_bias = d_gam = d_bet = None
    if not triv_bias:
        d_bias = nc.dram_tensor("bias_b", [128, D], F32, kind="ExternalInput")
    if not triv_gamma:
        d_gam = nc.dram_tensor("gamma_b", [128, D], F32, kind="ExternalInput")
    if not triv_beta:
        d_bet = nc.dram_tensor("beta_b", [128, D], F32, kind="ExternalInput")
    d_out = nc.dram_tensor("out", [128, nw, 2, D], BF16, kind="ExternalOutput")

    with tile.TileContext(nc) as tc, ExitStack() as ctx:
        cpool = ctx.enter_context(tc.tile_pool(name="const", bufs=1))
        gplo = ctx.enter_context(tc.tile_pool(name="glo", bufs=4))
        gphi = ctx.enter_context(tc.tile_pool(name="ghi", bufs=4))
        htp = ctx.enter_context(tc.tile_pool(name="htp", bufs=2))
        spool = ctx.enter_context(tc.tile_pool(name="sel", bufs=8))
        apool = ctx.enter_context(tc.tile_pool(name="aggts", bufs=4))
        xpool = ctx.enter_context(tc.tile_pool(name="xp", bufs=GLN + 2))
        sqpool = ctx.enter_context(tc.tile_pool(name="sqp", bufs=2))
        ypool = ctx.enter_context(tc.tile_pool(name="yp", bufs=3))
        stpool = ctx.enter_context(tc.tile_pool(name="stats", bufs=2))
        ppA = ctx.enter_context(
            tc.tile_pool(name="psA", bufs=3, space=bass.MemorySpace.PSUM)
        )
        ppB = ctx.enter_context(
            tc.tile_pool(name="psB", bufs=3, space=bass.MemorySpace.PSUM)
        )
        ppT = ctx.enter_context(
            tc.tile_pool(name="psT", bufs=2, space=bass.MemorySpace.PSUM)
        )

        nc.gpsimd.load_library(_mlp_library)

        def cload(dram, shape, dtype=BF16):
            t = cpool.tile(shape, dtype, tag=dram.name, name=dram.name + "_t")
            nc.sync.dma_start(t[:], dram[:])
            return t

        t_ixlo = cload(d_ixlo, [128, TL * 8], I16)
        t_ixhi = cload(d_ixhi, [128, TH * 8], I16)
        t_dllo = cload(d_dllo, [128, TL], F32)
        t_dlhi = cload(d_dlhi, [128, TH], F32)
        t_crlo = cload(d_crlo, [128, TL], F32)
        t_crhi = cload(d_crhi, [128, TH], F32)
        t_iota = cload(d_iota, [128, 128])
        t_ident = cload(d_ident, [128, 128])
        t_wsT = cload(d_wsT, [D, D])
        t_wmT = cload(d_wmT, [D, D])
        t_bias = None if triv_bias else cload(d_bias, [128, D], F32)
        t_gam = None if triv_gamma else cload(d_gam, [128, D], F32)
        t_bet = None if triv_beta else cload(d_bet, [128, D], F32)

        blocks = {"lo": [], "hi": []}
        issued = {"lo": 0, "hi": 0}
        totals = {"lo": TL, "hi": TH}
        gsrc = {"lo": d_hlo, "hi": d_hhi}
        gix = {"lo": t_ixlo, "hi": t_ixhi}
        gpool = {"lo": gplo, "hi": gphi}

        def ensure_gathered(kind, upto):
            while issued[kind] < min(upto, totals[kind]):
                t0 = issued[kind]
                t1 = min(t0 + GMAX, totals[kind])
                blk = gpool[kind].tile(
                    [128, GMAX, D2], BF16, tag="g" + kind, name="g" + kind
                )
                nc.gpsimd.dma_gather(
                    blk[:, 0 : t1 - t0, :],
                    gsrc[kind][:],
                    gix[kind][:, t0 * 8 : t1 * 8],
                    num_idxs=(t1 - t0) * 128,
                    num_idxs_reg=(t1 - t0) * 128,
                    elem_size=D2,
                )
                blocks[kind].append(blk)
                issued[kind] = t1

        ht_tiles = {}
        pending = []  # (w, x_tile) awaiting group LayerNorm stats
        xs_g = ssq_g = None

        def flush_group(wlist):
            nonlocal xs_g, ssq_g
            gw = len(wlist)
            mu = stpool.tile([128, 2 * GLN], F32, tag="mu", name="mu")
            nc.vector.tensor_scalar_mul(mu[:, 0 : 2 * gw], xs_g[:, 0 : 2 * gw], 1.0 / D)
            nmusq = stpool.tile([128, 2 * GLN], F32, tag="nmusq", name="nmusq")
            nc.vector.scalar_tensor_tensor(
                nmusq[:, 0 : 2 * gw],
                mu[:, 0 : 2 * gw],
                -1.0,
                mu[:, 0 : 2 * gw],
                op0=ALU.mult,
                op1=ALU.mult,
            )
            varv = stpool.tile([128, 2 * GLN], F32, tag="varv", name="varv")
            nc.vector.scalar_tensor_tensor(
                varv[:, 0 : 2 * gw],
                ssq_g[:, 0 : 2 * gw],
                1.0 / D,
                nmusq[:, 0 : 2 * gw],
                op0=ALU.mult,
                op1=ALU.add,
            )
            vrec = stpool.tile([128, 2 * GLN], F32, tag="vrec", name="vrec")
            nc.vector.reciprocal(vrec[:, 0 : 2 * gw], varv[:, 0 : 2 * gw])
            rstd = stpool.tile([128, 2 * GLN], F32, tag="rstd", name="rstd")
            nc.scalar.activation(rstd[:, 0 : 2 * gw], vrec[:, 0 : 2 * gw], ACTF.Sqrt)
            nmr = stpool.tile([128, 2 * GLN], F32, tag="nmr", name="nmr")
            nc.vector.scalar_tensor_tensor(
                nmr[:, 0 : 2 * gw],
                mu[:, 0 : 2 * gw],
                -1.0,
                rstd[:, 0 : 2 * gw],
                op0=ALU.mult,
                op1=ALU.mult,
            )
            w0 = wlist[0][0]
            ygrp = ypool.tile([128, GLN, 2, D], BF16, tag="y", name="ygrp")
            for wi, (w2, x2) in enumerate(wlist):
                for b in (0, 1):
                    c = wi * 2 + b
                    yt = ygrp[:, wi, b, :]
                    if not (triv_gamma and triv_beta):
                        yf = ypool.tile([128, D], F32, tag="yf", name="yf")
                        yt = yf[:]
                    nc.scalar.activation(
                        yt,
                        x2[:, b, :],
                        ACTF.Identity,
                        bias=nmr[:, c : c + 1],
                        scale=rstd[:, c : c + 1],
                    )
                    if not triv_gamma:
                        yg2 = ypool.tile([128, D], F32, tag="yg", name="yg2")
                        nc.vector.tensor_mul(yg2[:], yt, t_gam[:])
                        yt = yg2[:]
                    if not triv_beta:
                        nc.vector.tensor_add(ygrp[:, wi, b, :], yt, t_bet[:])
                    elif not triv_gamma:
                        nc.vector.tensor_copy(ygrp[:, wi, b, :], yt)
            nc.sync.dma_start(
                d_out[:, w0 : w0 + gw, :, :], ygrp[:, 0:gw, :, :]
            )

        for w in range(nw):
            tlc = int(T_low[w])
            thc = int(T_high[w])
            cl0, ch0 = int(cl[w]), int(ch[w])
            ensure_gathered("lo", cl0 + tlc)
            ensure_gathered("hi", ch0 + thc)

            ci = w // CHW
            want = [ci, ci + 1] if w % CHW >= CHW - 3 else [ci]
            for cli in want:
                if cli not in ht_tiles and cli * CHW < nw:
                    c0 = cli * CHW
                    c1 = min(c0 + CHW, nw)
                    tht2 = htp.tile([128, CHW, 2, D], BF16, tag="ht", name="ht")
                    nc.sync.dma_start(
                        tht2[:, 0 : c1 - c0, :, :], d_hT[:, c0:c1, :, :]
                    )
                    ht_tiles[cli] = tht2
            tht = ht_tiles[ci]
            k = w % CHW

            if w % GLN == 0:
                xs_g = stpool.tile([128, 2 * GLN], F32, tag="xs", name="xs")
                ssq_g = stpool.tile([128, 2 * GLN], F32, tag="ssq", name="ssq")
            gcol0 = 2 * (w % GLN)

            # S tiles: one-hot * 1/count, one fused DVE tensor_scalar per tile
            # (bf16 in/out + per-partition f32 scalars -> 4x DVE mode).
            ntile = tlc + thc
            S = spool.tile([128, ntile, 128], BF16, tag="S", name="S")
            tiles = [("lo", cl0 + kk, kk, t_dllo, t_crlo) for kk in range(tlc)] + [
                ("hi", ch0 + kk, tlc + kk, t_dlhi, t_crhi) for kk in range(thc)
            ]
            for kind, t, scol, t_dl, t_cr in tiles:
                nc.vector.tensor_scalar(
                    S[:, scol, :],
                    t_iota[:],
                    t_dl[:, t : t + 1],
                    t_cr[:, t : t + 1],
                    op0=ALU.is_equal,
                    op1=ALU.mult,
                )

            # aggT[f, dst] per batch, accumulated over edge tiles in PSUM.
            # Batch groups run sequentially so both fit one PSUM bank.
            psA = ppA.tile([128, 2, 128], F32, tag="psA", name="psA")
            for b in (0, 1):
                for j, (kind, t, scol, _dl, _cr) in enumerate(tiles):
                    blk = blocks[kind][t // GMAX]
                    slot = t % GMAX
                    nc.tensor.matmul(
                        psA[:, b, :],
                        blk[:, slot, b * D : (b + 1) * D],
                        S[:, scol, :],
                        start=j == 0,
                        stop=j == len(tiles) - 1,
                    )

            # PSUM -> SBUF bf16 copy of aggT on the (otherwise idle) ACT engine.
            aggTs = apool.tile([128, 2, 128], BF16, tag="aggTs", name="aggTs")
            nc.vector.tensor_copy(aggTs[:], psA[:])

            # h rows (residual) recovered on-chip: hrowT = transpose(hT).
            # Vector ops may read only ONE PSUM input (psB takes that slot),
            # so stage the transposed rows to SBUF, one copy per engine.
            hrT = ppT.tile([128, 2, D], BF16, tag="hrT", name="hrT")
            for b in (0, 1):
                nc.tensor.transpose(hrT[:, b, :], tht[:, k, b, :], t_ident[:])
            hrs = apool.tile([128, 2, D], BF16, tag="hrs", name="hrs")
            nc.scalar.activation(hrs[:], hrT[:], ACTF.Copy)

            # psB[node, fo] = h @ W_self.T + h_agg @ W_msg.T  (per batch)
            psB = ppB.tile([128, 2, 128], F32, tag="psB", name="psB")
            for b in (0, 1):
                nc.tensor.matmul(
                    psB[:, b, :], tht[:, k, b, :], t_wsT[:], start=True, stop=False
                )
                nc.tensor.matmul(
                    psB[:, b, :], aggTs[:, b, :], t_wmT[:], start=False, stop=True
                )

            # x = h + relu(psB [+ bias]); row-sum accum into the group tile.
            x = xpool.tile([128, 2, D], F32, tag="x", name="x")
            for b in (0, 1):
                if triv_bias:
                    pre = psB[:, b, :]
                else:
                    t1b = xpool.tile([128, D], F32, tag="t1b", name="t1b")
                    nc.vector.tensor_add(t1b[:], psB[:, b, :], t_bias[:])
                    pre = t1b[:]
                nc.vector.scalar_tensor_tensor(
                    x[:, b, :],
                    pre,
                    0.0,
                    hrs[:, b, :],
                    op0=ALU.max,
                    op1=ALU.add,
                    accum_out=xs_g[:, gcol0 + b : gcol0 + b + 1],
                )
            sq = sqpool.tile([128, 2, D], F32, tag="sq", name="sq")
            for b in (0, 1):
                nc.scalar.activation(
                    sq[:, b, :],
                    x[:, b, :],
                    ACTF.Square,
                    accum_out=ssq_g[:, gcol0 + b : gcol0 + b + 1],
                )
            pending.append((w, x))
            if w % GLN == GLN - 1 or w == nw - 1:
                flush_group(pending)
                pending = []
    nc.compile()
    return nc


def _make_in_maps(h, edge_index, W_self, W_msg, bias, gamma, beta, g, prep):
    B, N, D, NG, n_core, nw, half = _geometry(g)
    T_low, T_high, cl, ch, per_group = prep
    n_pad = nw * 128

    h = np.asarray(h, np.float32)
    # Interleaved dual-batch bf16 node table: row n = [h[0,n] | h[1,n]].
    h2 = np.concatenate([h[0], h[1]], axis=1).astype(NPBF)
    h2lo = np.ascontiguousarray(h2[:half])
    h2hi = np.ascontiguousarray(h2[half:])
    wsT = np.ascontiguousarray(np.asarray(W_self, np.float32).T.astype(NPBF))
    wmT = np.ascontiguousarray(np.asarray(W_msg, np.float32).T.astype(NPBF))
    iota = np.ascontiguousarray(
        np.broadcast_to(np.arange(128, dtype=np.float32)[None, :], (128, 128)).astype(
            NPBF
        )
    )
    ident = np.ascontiguousarray(np.eye(128, dtype=np.float32).astype(NPBF))
    trivial = _trivial_flags(bias, gamma, beta)
    bias_b = np.ascontiguousarray(
        np.broadcast_to(np.asarray(bias, np.float32)[None, :], (128, D))
    )
    gam_b = np.ascontiguousarray(
        np.broadcast_to(np.asarray(gamma, np.float32)[None, :], (128, D))
    )
    bet_b = np.ascontiguousarray(
        np.broadcast_to(np.asarray(beta, np.float32)[None, :], (128, D))
    )

    # Pad-row pattern keeps pad-row LayerNorm variance bounded away from 0.
    padrow = (0.001 * (1.0 - 2.0 * (np.arange(D) % 2))).astype(np.float32)

    in_maps = []
    for c in range(NG):
        pg = per_group[c]
        perm = pg["perm"]
        valid = perm >= 0
        hpad = np.empty((B, n_pad, D), np.float32)
        hpad[:, :] = padrow[None, None, :]
        for b in range(B):
            hpad[b][valid] = h[b][perm[valid]]
        # hT[p=f, w, b, n]
        hp = hpad.reshape(B, nw, 128, D)
        hT = np.ascontiguousarray(hp.transpose(3, 1, 0, 2).astype(NPBF))
        m = {
            "h2lo": h2lo,
            "h2hi": h2hi,
            "ixlo": pg["ixlo"],
            "ixhi": pg["ixhi"],
            "dllo": pg["dllo"],
            "dlhi": pg["dlhi"],
            "crlo": pg["crlo"],
            "crhi": pg["crhi"],
            "hT": hT,
            "wsT": wsT,
            "wmT": wmT,
            "iota": iota,
            "ident": ident,
        }
        if not trivial[0]:
            m["bias_b"] = bias_b
        if not trivial[1]:
            m["gamma_b"] = gam_b
        if not trivial[2]:
            m["beta_b"] = bet_b
        in_maps.append(m)
    return in_maps


def _trivial_flags(bias, gamma, beta):
    return (
        not np.any(np.asarray(bias)),
        bool(np.all(np.asarray(gamma) == 1.0)),
        not np.any(np.asarray(beta)),
    )


def _decode_out(arr, perm, out, nw):
    # arr: [128, nw, 2, D] bf16; row (w*128+p) of batch b = arr[p, w, b, :].
    valid = perm >= 0
    a = np.asarray(arr).transpose(1, 0, 2, 3).reshape(nw * 128, 2, -1)
    af = a[valid].astype(np.float32)
    out[0, perm[valid]] = af[:, 0]
    out[1, perm[valid]] = af[:, 1]


def kernel(h, edge_index, W_self, W_msg, bias, gamma, beta):
    global LAST_RESULTS
    g = FULL_GEO
    B, N, D, NG, n_core, nw, half = _geometry(g)
    prep = _preprocess(edge_index, g)
    T_low, T_high, cl, ch, per_group = prep
    trivial = _trivial_flags(bias, gamma, beta)
    nc = _build_program(g, T_low, T_high, cl, ch, trivial)
    in_maps = _make_in_maps(h, edge_index, W_self, W_msg, bias, gamma, beta, g, prep)
    res = run_bass_kernel_spmd(nc, in_maps, core_ids=list(range(NG)))
    LAST_RESULTS = res
    out = np.empty((B, N, D), np.float32)
    for c in range(NG):
        _decode_out(res.results[c]["out"], per_group[c]["perm"], out, nw)
    return out
